# revision 61
# baseline (speedup 1.0000x reference)
"""Causal self-attention (B=4, T=2048, C=2048, H=16, rope) on 8 trn2 cores.

Sharding: core c handles batch b = c//2 and head-group g = c%2 (8 heads).

Flash-style chunk pipeline, q/k/v SBUF-resident in bf16 (no DRAM round
trip).  T is split into 4 chunks of 512 tokens; per chunk:

  B(ch): qkv for the chunk.  q/k via W-stationary matmuls (N=512) with
    rope fused on DVE straight out of PSUM (sign-vector trick), written
    to SBUF bf16.  v via x-stationary matmuls so it lands pre-transposed
    [t, d] with a ones column appended for the softmax denominator.
  C(ch): causal attention of q-chunk ch over k/v chunks 0..ch per head:
    per 128-token k-block j: scoresT = k_j^T q (bf16, shrunk to the
    causal q-range for diagonal blocks), exp on ACT -> bf16 block tile,
    triangular mask multiply on the exact-diagonal 128x128 tile only,
    attn@V with the ones column, per-partition reciprocal normalize,
    PE-transpose y to [d, t].
  D(ch): per half (4 heads) pairwise AllGather of the y chunk; at ch=3
    the second half goes in per-head eighth-collectives so the final
    exchange is minimal.
  E(ch): out[t-chunk] = y^T Wp^T in bf16; all 16 head-tiles come back
    from the AllGather output (replica-indexed, so the program is
    core-independent).  For ch=3 each PSUM chain is split: 12 early
    matmuls (heads 0-5) run before the last exchanges complete, parked
    in SBUF; the last 4 (heads 6,7) land in a fresh bank and are added
    back on DVE.

Emission interleaves B(ch+1) | C(ch) | E(ch-1) weighted by per-unit
PE-time.  Weight/x loads are peeled into separate weave units so every
DMA is issued one unit ahead, and each load class has a dedicated
engine DMA ring (qk->sync, x->scalar/gpsimd, wv->tensor,
yf/wp/out->vector) so loads never queue behind each other.
"""
import sys

sys.path.insert(0, "/opt/trn_rl_repo")

import numpy as np
import ml_dtypes

import concourse.bass as bass
import concourse.tile as tile
from concourse import bacc, mybir
from concourse import bass_utils

F32 = mybir.dt.float32
BF16 = mybir.dt.bfloat16
AF = mybir.ActivationFunctionType
ALU = mybir.AluOpType
BF16NP = ml_dtypes.bfloat16

B, T, C = 4, 2048, 2048
NH, D = 16, 128
HL = 8              # heads per core
NCT = C // 128      # 16 c-tiles
NCH = 4             # 512-token chunks
SCALE = 1.0 / np.sqrt(D)
RG = [[0, 1], [2, 3], [4, 5], [6, 7]]


def _weave(*streams):
    """streams: lists of (fn, weight). Emit round-robin by weighted progress."""
    streams = [s for s in streams if s]
    totals = [sum(w for _, w in s) or 1.0 for s in streams]
    done = [0.0] * len(streams)
    idx = [0] * len(streams)
    while True:
        best, bestv = -1, None
        for i, s in enumerate(streams):
            if idx[i] < len(s):
                v = done[i] / totals[i]
                if bestv is None or v < bestv:
                    best, bestv = i, v
        if best < 0:
            break
        fn, w = streams[best][idx[best]]
        fn()
        done[best] += w
        idx[best] += 1


def _build():
    nc = bacc.Bacc("TRN2", target_bir_lowering=False, debug=False, num_devices=8)
    xb = nc.dram_tensor("xb", [128, NCH, 4, 4, 512], BF16, kind="ExternalInput").ap()
    Wqk = nc.dram_tensor("Wqk", [16, 128, C], BF16, kind="ExternalInput").ap()
    WvT = nc.dram_tensor("WvT", [128, 2, NCT, 512], BF16, kind="ExternalInput").ap()
    Wp2 = nc.dram_tensor("Wp2", [128, 2, 2, 8, 512], BF16, kind="ExternalInput").ap()
    cos2 = nc.dram_tensor("cos2", [128, T], BF16, kind="ExternalInput").ap()
    sin1 = nc.dram_tensor("sin1", [64, T], BF16, kind="ExternalInput").ap()
    sgn = nc.dram_tensor("sgn", [128, 1], F32, kind="ExternalInput").ap()
    m4d = nc.dram_tensor("m4d", [128, 128], BF16, kind="ExternalInput").ap()
    ident = nc.dram_tensor("ident", [128, 128], BF16, kind="ExternalInput").ap()
    out = nc.dram_tensor("out", [T, C // 2], F32, kind="ExternalOutput").ap()

    with tile.TileContext(nc) as tc:
        with tc.tile_pool(name="dram", bufs=1, space="DRAM") as dram, \
             tc.tile_pool(name="const", bufs=1) as cpool:
            ygi = [dram.tile([128, 4, 512], BF16, name=f"ygi{i}") for i in range(7)]
            ygo = [dram.tile([2, 128, 4, 512], BF16, name=f"ygo{i}")
                   for i in range(7)]
            # ch3 exchanges: one quarter (2-head) collective per head pair,
            # staged per-head as each head's attention completes
            ygi3 = [dram.tile([128, 2, 512], BF16, name=f"ygi3_{i}")
                    for i in range(4)]
            ygo3 = [dram.tile([2, 128, 2, 512], BF16, name=f"ygo3_{i}")
                    for i in range(4)]

            xpool = tc.alloc_tile_pool(name="xp", bufs=6)
            wqkp = tc.alloc_tile_pool(name="wqkp", bufs=3)
            wvp = tc.alloc_tile_pool(name="wvp", bufs=1)
            wpp = tc.alloc_tile_pool(name="wpp", bufs=2)
            kp = tc.alloc_tile_pool(name="kp", bufs=32)
            qp = tc.alloc_tile_pool(name="qp", bufs=8)
            vap = tc.alloc_tile_pool(name="vap", bufs=8)
            ebp = tc.alloc_tile_pool(name="ebp", bufs=17)
            yp_ = tc.alloc_tile_pool(name="ypl", bufs=2)
            yfp = tc.alloc_tile_pool(name="yfp", bufs=3)
            abp = tc.alloc_tile_pool(name="abp", bufs=1)
            ynp = tc.alloc_tile_pool(name="ynp", bufs=2)
            op_ = tc.alloc_tile_pool(name="op", bufs=2)
            ps1 = tc.alloc_tile_pool(name="ps1", bufs=2, space="PSUM")
            spp = tc.alloc_tile_pool(name="spp", bufs=4, space="PSUM")
            ypp = tc.alloc_tile_pool(name="ypp", bufs=2, space="PSUM")

            kts = {}   # (h, ch) -> k tile [128, 512]
            qts = {}   # (h, ch) -> q tile [128, 512]
            vas = {}   # (ch, hf) -> va tile [128, 4, 4, 129] (tt, h4, d+1)
            xts = {}   # (ch, qq) -> x tile [128, 4, 512]
            ys = {}    # (ch, hf) -> y tile [128, 4, 512]
            yfs = {}   # (ch, hf) -> yf tile [128, 2, 4, 512] (replica, h4, t)
            wts = {}   # (ch, h, kq) -> prefetched qk weight tile
            wvcur = {}
            wpcur = {}
            ebs = {h: [] for h in range(HL)}
            prts = {}  # (fc, tt) -> partial e3 tile [128, 512] bf16

            m4_sb = cpool.tile([128, 128], BF16)
            id_sb = cpool.tile([128, 128], BF16)
            sg_sb = cpool.tile([128, 1], F32)
            c2_sb = cpool.tile([128, T], BF16)
            s1_sb = cpool.tile([64, T], BF16)

            def load_small_consts():
                nc.gpsimd.dma_start(sg_sb[:], sgn)
                nc.gpsimd.dma_start(m4_sb[:], m4d)
                nc.gpsimd.dma_start(id_sb[:], ident)

            def load_rope_consts():
                nc.scalar.dma_start(c2_sb[:], cos2)
                nc.gpsimd.dma_start(s1_sb[:], sin1)

            def load_x(ch, qq):
                def go():
                    xt = xpool.tile([128, 4, 512], BF16, name="xt")
                    nc.scalar.dma_start(xt[:], xb[:, ch, qq])
                    xts[(ch, qq)] = xt
                return go

            def qk_load(ch, h, kq):
                def go():
                    wt = wqkp.tile([128, C], BF16, name="wt")
                    nc.sync.dma_start(wt[:], Wqk[h * 2 + kq])
                    wts[(ch, h, kq)] = wt
                return go

            def rope_store(ps, ch, h, kq):
                """Rope + bf16 store of a finished q/k PSUM tile."""
                t0 = ch * 512
                a_t = abp.tile([128, 512], BF16, name="a_t")
                nc.vector.tensor_mul(a_t[:], ps[:], c2_sb[:, t0:t0 + 512])
                b_t = abp.tile([128, 512], BF16, name="b_t")
                nc.vector.tensor_mul(
                    b_t[0:64, :], ps[64:128, :], s1_sb[:, t0:t0 + 512])
                nc.vector.tensor_mul(
                    b_t[64:128, :], ps[0:64, :], s1_sb[:, t0:t0 + 512])
                if kq == 0:
                    dst = qp.tile([128, 512], BF16, name="qt")
                    qts[(h, ch)] = dst
                else:
                    dst = kp.tile([128, 512], BF16, name="kt")
                    kts[(h, ch)] = dst
                nc.vector.scalar_tensor_tensor(
                    dst[:], b_t[:], sg_sb[:], a_t[:],
                    op0=ALU.mult, op1=ALU.add)

            def qk_comp(ch, h, kq):
                def go():
                    wt = wts.pop((ch, h, kq))
                    ps = ps1.tile([128, 512], F32, name="ps")
                    for ct in range(NCT):
                        nc.tensor.matmul(
                            ps[:], wt[:, ct * 128:(ct + 1) * 128],
                            xts[(ch, ct // 4)][:, ct % 4, :],
                            start=(ct == 0), stop=(ct == NCT - 1))
                    rope_store(ps, ch, h, kq)
                return go

            def qk_startup():
                """First 4 qk units (h0/h1 x q/k) as x-quarter-interleaved
                waves: 4 open PSUM chains (idle spp pool) so every arriving
                x quarter immediately feeds 16 matmuls.  Loads are spread
                over all three rings in need order; full-tile weight loads
                keep 4KB DMA descriptors (column-sliced loads run 4x
                slower per byte)."""
                # x quarter 0 split by c-tile halves on two rings
                xt0 = xpool.tile([128, 4, 512], BF16, name="xt")
                xts[(0, 0)] = xt0
                w = {}
                w[0] = wqkp.tile([128, C], BF16, name="wt")
                nc.sync.dma_start(w[0][:], Wqk[0])
                w[1] = wqkp.tile([128, C], BF16, name="wt")
                nc.sync.dma_start(w[1][:], Wqk[1])
                nc.scalar.dma_start(xt0[:, 0:2, :], xb[:, 0, 0, 0:2])
                nc.sync.dma_start(xt0[:, 2:4, :], xb[:, 0, 0, 2:4])
                w[2] = wqkp.tile([128, C], BF16, name="wt")
                nc.scalar.dma_start(w[2][:], Wqk[2])
                w[3] = xpool.tile([128, 4, 512], BF16, name="xt")
                nc.scalar.dma_start(w[3][:], Wqk[3])
                for qq, eng in ((1, nc.sync), (2, nc.scalar), (3, nc.sync)):
                    xt = xpool.tile([128, 4, 512], BF16, name="xt")
                    eng.dma_start(xt[:], xb[:, 0, qq])
                    xts[(0, qq)] = xt
                load_rope_consts()
                psums = [spp.tile([128, 512], F32, name="sp") for _ in range(4)]

                def wsl(u, ct):
                    if u == 3:   # [128, 4, 512]-shaped borrow
                        return w[3][:, ct // 4, (ct % 4) * 128:(ct % 4 + 1) * 128]
                    return w[u][:, ct * 128:(ct + 1) * 128]

                for p in range(4):
                    for u in range(4):
                        for c4 in range(4):
                            ct = p * 4 + c4
                            nc.tensor.matmul(
                                psums[u][:], wsl(u, ct),
                                xts[(0, p)][:, c4, :],
                                start=(ct == 0), stop=(ct == NCT - 1))
                for u in range(4):
                    rope_store(psums[u], 0, u // 2, u % 2)

            def wv_load(hf):
                def go():
                    wv = wvp.tile([128, NCT, 512], BF16, name="wv")
                    nc.scalar.dma_start(wv[:], WvT[:, hf])
                    wvcur[0] = wv
                return go

            def v_unit(ch, hf, tt):
                def go():
                    if tt == 0:
                        va = vap.tile([128, 4, 4, 129], BF16, name="va")
                        nc.vector.memset(va[:, :, :, 128:129], 1.0)
                        vas[(ch, hf)] = va
                    wv = wvcur[0]
                    va = vas[(ch, hf)]
                    vps = ps1.tile([128, 512], F32, name="ps")
                    for ct in range(NCT):
                        nc.tensor.matmul(
                            vps[:],
                            xts[(ch, ct // 4)][:, ct % 4, tt * 128:(tt + 1) * 128],
                            wv[:, ct, :],
                            start=(ct == 0), stop=(ct == NCT - 1))
                    nc.scalar.copy(
                        va[:, tt, :, 0:128],
                        vps.rearrange("p (h d) -> p h d", d=128))
                return go

            def s_unit(ch, h, j):
                """Scores for 128-token k-block j (global) vs q-chunk ch,
                shrunk to the causal q-range for diagonal blocks."""
                def go():
                    jb = j - 4 * ch          # >=0 only for diagonal blocks
                    qlo = max(0, jb) * 128
                    sp = spp.tile([128, 512], F32, name="sp")
                    nc.tensor.matmul(
                        sp[:, qlo:512],
                        kts[(h, j // 4)][:, (j % 4) * 128:(j % 4 + 1) * 128],
                        qts[(h, ch)][:, qlo:512],
                        start=True, stop=True)
                    eb = ebp.tile([128, 512], BF16, name="eb")
                    nc.scalar.activation(
                        eb[:, qlo:512], sp[:, qlo:512], AF.Exp,
                        scale=float(SCALE))
                    if jb >= 0:
                        nc.vector.tensor_mul(
                            eb[:, qlo:qlo + 128], eb[:, qlo:qlo + 128], m4_sb[:])
                    ebs[h].append(eb)
                return go

            def a_unit(ch, h, ql):
                def go():
                    yt = ypp.tile([128, 129], F32, name="yp", tag="yp")
                    jmax = 4 * ch + ql
                    for j in range(jmax + 1):
                        nc.tensor.matmul(
                            yt[:],
                            ebs[h][j][:, ql * 128:(ql + 1) * 128],
                            vas[(j // 4, h // 4)][:, j % 4, h % 4, :],
                            start=(j == 0), stop=(j == jmax))
                    rc = ynp.tile([128, 1], F32, name="rc")
                    nc.vector.reciprocal(rc[:], yt[:, 128:129])
                    yn = ynp.tile([128, 128], BF16, name="yn")
                    nc.vector.tensor_scalar_mul(yn[:], yt[:, 0:128], rc[:])
                    ytp = ypp.tile([128, 128], BF16, name="ytp", tag="yp")
                    nc.tensor.transpose(ytp[:], yn[:], id_sb[:])
                    nc.vector.tensor_copy(
                        ys[(ch, h // 4)][:, h % 4, ql * 128:(ql + 1) * 128],
                        ytp[:])
                return go

            def y_alloc(ch, hf):
                def go():
                    ys[(ch, hf)] = yp_.tile([128, 4, 512], BF16, name="yc")
                return go

            def d_unit(ch, hf):
                def go():
                    i = ch * 2 + hf
                    nc.gpsimd.dma_start(ygi[i], ys[(ch, hf)][:])
                    nc.gpsimd.collective_compute(
                        "AllGather", ALU.bypass,
                        ins=[ygi[i][:].opt()], outs=[ygo[i][:].opt()],
                        replica_groups=RG)
                return go

            def yf_load(ch, hf):
                def go():
                    yf = yfp.tile([128, 2, 4, 512], BF16, name="yf")
                    nc.gpsimd.dma_start(
                        yf[:], ygo[ch * 2 + hf].rearrange("r p h t -> p r h t"))
                    yfs[(ch, hf)] = yf
                return go

            t31 = {}   # (head, r) -> [128, 512] gathered-ch3-y tile (kp-parked)

            def d3_piece(h):
                """Stage head h's ch3 y into its pair collective input as
                soon as that head's attention finishes."""
                def go():
                    nc.gpsimd.dma_start(
                        ygi3[h // 2][:, h % 2:h % 2 + 1, :],
                        ys[(3, h // 4)][:, h % 4:h % 4 + 1, :])
                return go

            y30 = {}   # hp -> [128, 2, 2, 512] gathered tile (heads 0-3)

            def d3_pair(hp):
                """Exchange ch3 heads (2*hp, 2*hp+1).  Pairs 0,1 (heads
                0-3, consumed by every e3 chain early) land in small yfp
                tiles; pairs 2,3 land in per-(head, r) kp-parked tiles so
                the late chains dep on exactly the data they read."""
                def go():
                    nc.gpsimd.collective_compute(
                        "AllGather", ALU.bypass,
                        ins=[ygi3[hp][:].opt()], outs=[ygo3[hp][:].opt()],
                        replica_groups=RG)
                    if hp < 2:
                        if hp == 0:
                            y30[0] = yfp.tile([128, 2, 4, 512], BF16,
                                              name="yf")
                        nc.gpsimd.dma_start(
                            y30[0][:, :, 2 * hp:2 * hp + 2, :],
                            ygo3[hp].rearrange("r p h t -> p r h t"))
                    else:
                        # pair 3 gates the kernel tail: spread its four
                        # gathered-head loads over all three rings
                        rr = ([nc.gpsimd] * 4 if hp == 2 else
                              [nc.gpsimd, nc.sync, nc.scalar, nc.sync])
                        for i, (hh, r) in enumerate(
                                [(hh, r) for hh in range(2) for r in range(2)]):
                            t = kp.tile([128, 512], BF16, name="kt")
                            rr[i].dma_start(t[:], ygo3[hp][r, :, hh, :])
                            t31[(2 * hp + hh, r)] = t
                return go

            def wp_load(ch, fc):
                """Prefetch proj weights for (ch, fc). For the final
                (ch=3, fc=1) borrow idle x-pool slots so the load never
                waits on fc0's readers."""
                def go():
                    if ch == 3 and fc == 1:
                        for r in range(2):
                            for hh in range(2):
                                wph = xpool.tile([128, 4, 512], BF16, name="xt")
                                (nc.scalar if hh == 0 else nc.gpsimd).dma_start(
                                    wph[:], Wp2[:, r, fc, 4 * hh:4 * hh + 4])
                                wpcur[(fc, r, hh)] = wph
                    else:
                        for r in range(2):
                            wp = wpp.tile([128, 8, 512], BF16, name="wp")
                            nc.gpsimd.dma_start(wp[:], Wp2[:, r, fc])
                            wpcur[(fc, r, 0)] = wp[:, 0:4, :]
                            wpcur[(fc, r, 1)] = wp[:, 4:8, :]
                return go

            def e_unit(ch, fc, tt):
                def go():
                    pp = ps1.tile([128, 512], F32, name="ps")
                    srcs = [(hf, r, h4) for hf in range(2) for r in range(2)
                            for h4 in range(4)]
                    for i, (hf, r, h4) in enumerate(srcs):
                        wp = wpcur[(fc, r, hf)]
                        nc.tensor.matmul(
                            pp[:],
                            yfs[(ch, hf)][:, r, h4, tt * 128:(tt + 1) * 128],
                            wp[:, h4, :],
                            start=(i == 0), stop=(i == 15))
                    ob = op_.tile([128, 512], F32, name="ob")
                    nc.vector.tensor_copy(ob[:], pp[:])
                    t0 = ch * 512 + tt * 128
                    nc.scalar.dma_start(
                        out[t0:t0 + 128, fc * 512:(fc + 1) * 512], ob[:])
                return go

            def e3_early(fc, tt):
                """First 12 matmuls of the (3, fc, tt) chain: heads 0-5
                (hf0 fully + hf1 h4 in {0,1}), parked in SBUF bf16."""
                def go():
                    pp = ps1.tile([128, 512], F32, name="ps")
                    for i, (r, h4) in enumerate(
                            [(r, h4) for r in range(2) for h4 in range(4)]):
                        nc.tensor.matmul(
                            pp[:],
                            y30[0][:, r, h4, tt * 128:(tt + 1) * 128],
                            wpcur[(fc, r, 0)][:, h4, :],
                            start=(i == 0), stop=False)
                    for i, (h4, r) in enumerate(
                            [(h4, r) for h4 in range(2) for r in range(2)]):
                        nc.tensor.matmul(
                            pp[:],
                            t31[(4 + h4, r)][:, tt * 128:(tt + 1) * 128],
                            wpcur[(fc, r, 1)][:, h4, :],
                            start=False, stop=(i == 3))
                    # park the partial in a dead chunk-0 k-tile slot
                    pt = kp.tile([128, 512], BF16, name="kt")
                    nc.vector.tensor_copy(pt[:], pp[:])
                    prts[(fc, tt)] = pt
                return go

            def e3_late(fc, tt, ring):
                """Last 4 matmuls (heads 6,7) + add-back + store."""
                def go():
                    pp = ps1.tile([128, 512], F32, name="ps")
                    srcs = [(r, h4) for h4 in range(2, 4) for r in range(2)]
                    for i, (r, h4) in enumerate(srcs):
                        nc.tensor.matmul(
                            pp[:],
                            t31[(4 + h4, r)][:, tt * 128:(tt + 1) * 128],
                            wpcur[(fc, r, 1)][:, h4, :],
                            start=(i == 0), stop=(i == 3))
                    ob = op_.tile([128, 512], F32, name="ob")
                    nc.vector.scalar_tensor_tensor(
                        ob[:], pp[:], 1.0, prts[(fc, tt)][:],
                        op0=ALU.mult, op1=ALU.add)
                    t0 = 3 * 512 + tt * 128
                    ring.dma_start(
                        out[t0:t0 + 128, fc * 512:(fc + 1) * 512], ob[:])
                return go

            def b_stream(ch, with_xl=None, skip_h01=False):
                """qkv units with loads peeled one unit ahead."""
                pairs = []   # (load_fn_or_None, comp_fn, weight)
                if not skip_h01:
                    for h in (0, 1):
                        for kq in (0, 1):
                            pairs.append(
                                (qk_load(ch, h, kq), qk_comp(ch, h, kq), 3.4))
                pairs.append((wv_load(0), v_unit(ch, 0, 0), 3.4))
                for tt in range(1, 4):
                    pairs.append((None, v_unit(ch, 0, tt), 3.4))
                for h in (2, 3):
                    for kq in (0, 1):
                        pairs.append((qk_load(ch, h, kq), qk_comp(ch, h, kq), 3.4))
                pairs.append((wv_load(1), v_unit(ch, 1, 0), 3.4))
                for tt in range(1, 4):
                    pairs.append((None, v_unit(ch, 1, tt), 3.4))
                for h in (4, 5, 6, 7):
                    for kq in (0, 1):
                        pairs.append((qk_load(ch, h, kq), qk_comp(ch, h, kq), 3.4))
                u = []
                # emit load i+1 before comp i (1-unit lookahead)
                if pairs[0][0] is not None:
                    u.append((pairs[0][0], 0.05))
                for i, (_, comp, w) in enumerate(pairs):
                    nxt = pairs[i + 1][0] if i + 1 < len(pairs) else None
                    if nxt is not None:
                        u.append((nxt, 0.05))
                    u.append((comp, w))
                if with_xl is not None:
                    for qi in range(4):
                        u.append((load_x(with_xl, qi), 0.1))
                return u

            def c_head(ch, h):
                """Units for one head of c_stream(ch)."""
                u = []
                if h % 4 == 0:
                    u.append((y_alloc(ch, h // 4), 0.05))

                def reset(h=h):
                    ebs[h] = []
                u.append((reset, 0.0))
                for j in range(4 * ch + 4):
                    jb = j - 4 * ch
                    w = 0.22 * (512 - max(0, jb) * 128) / 512.0
                    u.append((s_unit(ch, h, j), w))
                for ql in range(4):
                    u.append((a_unit(ch, h, ql), 0.4 + (4 * ch + ql) * 0.066))
                return u

            def c_stream(ch):
                """Attention for chunk ch; also prefetches the e-stage's
                fc0 weights and the first gathered-y half as soon as the
                h0-3 collective has had time to complete."""
                u = []
                for h in range(8):
                    u += c_head(ch, h)
                    if h == 3:
                        u.append((d_unit(ch, 0), 0.1))
                    elif h == 5:
                        u.append((wp_load(ch, 0), 0.1))
                        u.append((yf_load(ch, 0), 0.1))
                    elif h == 7:
                        u.append((d_unit(ch, 1), 0.1))
                return u

            def e_stream(ch):
                u = [(yf_load(ch, 1), 0.1)]
                for tt in range(4):
                    u.append((e_unit(ch, 0, tt), 3.5))
                u.append((wp_load(ch, 1), 0.1))
                for tt in range(4):
                    u.append((e_unit(ch, 1, tt), 3.5))
                return u

            # ---------------- emit ----------------
            load_small_consts()
            qk_startup()

            for fn, _w in b_stream(0, with_xl=1, skip_h01=True):
                fn()

            _weave(b_stream(1, with_xl=2), c_stream(0))
            _weave(b_stream(2, with_xl=3), c_stream(1), e_stream(0))
            _weave(b_stream(3), c_stream(2), e_stream(1))

            # phase 4: c3 heads 0-5 with e_stream(2) + e3 weight loads.
            # Each head's y is staged into its pair-collective input as
            # soon as it finishes; pair exchanges fire at h1/h3/h5 so the
            # CC pipeline drains long before the e3 chains need the data.
            c3_p1 = []
            for h in range(6):
                c3_p1 += c_head(3, h)
                c3_p1.append((d3_piece(h), 0.05))
                if h % 2 == 1:
                    c3_p1.append((d3_pair(h // 2), 0.05))
            e2_plus = e_stream(2)
            e2_plus.append((wp_load(3, 0), 0.1))
            e2_plus.append((wp_load(3, 1), 0.1))
            _weave(c3_p1, e2_plus)

            # phase 5: c3 heads 6,7 run at full priority (so the final
            # exchange fires ASAP); only 3 early-e3 chains are woven in
            # to fill the exp-bound pockets, rest run after.
            c3_p2 = []
            c3_p2 += c_head(3, 6)
            c3_p2.append((d3_piece(6), 0.05))
            c3_p2 += c_head(3, 7)
            c3_p2.append((d3_piece(7), 0.05))
            c3_p2.append((d3_pair(3), 0.05))
            e3e = [(e3_early(fc, tt), 2.6) for fc in range(2) for tt in range(4)]
            _weave(c3_p2, e3e[0:3])
            for fn, w in e3e[3:]:
                fn()

            # tail: last 4 matmuls per chain + add-back, outs spread on rings
            rings = [nc.sync, nc.scalar, nc.gpsimd]
            i = 0
            for fc in range(2):
                for tt in range(4):
                    e3_late(fc, tt, rings[i % 3])()
                    i += 1

            for p in [ypp, spp, ps1, op_, ynp, abp, yfp, yp_, ebp, vap,
                      qp, kp, wpp, wvp, wqkp, xpool]:
                p.release()
    nc.compile()
    return nc


_NC = None


def _get_nc():
    global _NC
    if _NC is None:
        _NC = _build()
    return _NC


def _rope_tables():
    inv_freq = (1.0 / (10000.0 ** (np.arange(0, D, 2, dtype=np.float32) / D)))
    t = np.arange(T, dtype=np.float32)
    freqs = np.outer(t, inv_freq).astype(np.float32)      # [T, 64]
    cos = np.cos(freqs).T                                 # [64, T]
    sin = np.sin(freqs).T
    return cos, sin


def _tile_w(Wt):
    """[128 r, 2048 c] weight tile -> [128 c_lo, 2048 (ct r)] layout."""
    return np.ascontiguousarray(
        Wt.T.reshape(NCT, 128, 128).transpose(1, 0, 2).reshape(128, C))


def make_in_maps(x, W_attn, W_proj):
    perm = np.concatenate([np.arange(0, D, 2), np.arange(1, D, 2)])
    cos, sin = _rope_tables()
    cos2 = np.concatenate([cos, cos], 0).astype(BF16NP)
    sin1 = np.ascontiguousarray(sin).astype(BF16NP)
    sgn = np.concatenate([-np.ones((64, 1)), np.ones((64, 1))]).astype(np.float32)
    p_i = np.arange(128)[:, None]
    c_i = np.arange(128)[None, :]
    m4d_ = (c_i >= p_i).astype(BF16NP)

    xbf = x.astype(BF16NP)
    in_maps = []
    for core in range(8):
        b, g = core // 2, core % 2
        # xb [128 p, ch, qq, ct4, 512 t] = x[b, ch*512+t, (qq*4+ct4)*128+p]
        xt = np.ascontiguousarray(xbf[b].T)               # [C, T]
        xb_ = xt.reshape(4, 4, 128, NCH, 512).transpose(2, 3, 0, 1, 4)
        # q/k weight tiles, rope-permuted; order [h0 q, h0 k, h1 q, ...]
        wtiles = []
        for h in range(HL):
            hg = g * HL + h
            wtiles.append(_tile_w(W_attn[hg * D:(hg + 1) * D][perm]))
            wtiles.append(_tile_w(W_attn[C + hg * D:C + (hg + 1) * D][perm]))
        Wqk_ = np.stack(wtiles, 0).astype(BF16NP)
        # WvT [128 p, half, ct, 512 dv] = Wv[g*1024 + hf*512 + dv, ct*128 + p]
        wv = W_attn[2 * C + g * 1024:2 * C + (g + 1) * 1024]   # [1024 dv, C]
        WvT_ = np.ascontiguousarray(
            wv.reshape(2, 512, NCT, 128).transpose(3, 0, 2, 1)).astype(BF16NP)
        # Wp2 [128 p, r, fc, 8, 512 f] = Wp[g*1024 + fc*512 + f, (r*8+i8)*128+p]
        wp = W_proj[g * 1024:(g + 1) * 1024]                   # [1024 f, C]
        wp_t = wp.reshape(2, 512, NCT, 128)                    # [fc, f, ct, p]
        Wp2_ = np.stack([wp_t[:, :, 0:8], wp_t[:, :, 8:16]], 0)  # [r, fc, f, 8, p]
        Wp2_ = np.ascontiguousarray(Wp2_.transpose(4, 0, 1, 3, 2)).astype(BF16NP)
        in_maps.append({
            "xb": np.ascontiguousarray(xb_),
            "Wqk": Wqk_,
            "WvT": WvT_,
            "Wp2": Wp2_,
            "cos2": cos2, "sin1": sin1, "sgn": sgn,
            "m4d": m4d_, "ident": np.eye(128, dtype=BF16NP),
        })
    return in_maps


def _assemble(results):
    out = np.empty((B, T, C), dtype=np.float32)
    for core in range(8):
        b, g = core // 2, core % 2
        out[b][:, g * (C // 2):(g + 1) * (C // 2)] = results[core]["out"]
    return out


def run(x, W_attn, W_proj, **spmd_kwargs):
    nc = _get_nc()
    in_maps = make_in_maps(np.asarray(x, dtype=np.float32),
                           np.asarray(W_attn, dtype=np.float32),
                           np.asarray(W_proj, dtype=np.float32))
    res = bass_utils.run_bass_kernel_spmd(
        nc, in_maps, core_ids=list(range(8)), **spmd_kwargs)
    return _assemble(res.results), res


def kernel(x, W_attn, W_proj):
    out, _ = run(x, W_attn, W_proj)
    return out


# revision 62
# speedup vs baseline: 1.3002x; 1.3002x over previous
"""Causal self-attention (B=4, T=2048, C=2048, H=16, rope) on 8 trn2 cores.

Sharding: core c handles batch b = c//2 and head-group g = c%2 (8 heads).

Flash-style chunk pipeline, q/k/v SBUF-resident in bf16 (no DRAM round
trip).  T is split into 4 chunks of 512 tokens; per chunk:

  B(ch): qkv for the chunk.  q/k via W-stationary matmuls (N=512) with
    rope fused on DVE straight out of PSUM (sign-vector trick), written
    to SBUF bf16.  v via x-stationary matmuls so it lands pre-transposed
    [t, d] with a ones column appended for the softmax denominator.
  C(ch): causal attention of q-chunk ch over k/v chunks 0..ch per head:
    per 128-token k-block j: scoresT = k_j^T q (bf16, shrunk to the
    causal q-range for diagonal blocks), exp on ACT -> bf16 block tile,
    triangular mask multiply on the exact-diagonal 128x128 tile only,
    attn@V with the ones column, per-partition reciprocal normalize,
    PE-transpose y to [d, t].
  D(ch): per half (4 heads) pairwise AllGather of the y chunk; at ch=3
    the second half goes in per-head eighth-collectives so the final
    exchange is minimal.
  E(ch): out[t-chunk] = y^T Wp^T in bf16; all 16 head-tiles come back
    from the AllGather output (replica-indexed, so the program is
    core-independent).  For ch=3 each PSUM chain is split: 12 early
    matmuls (heads 0-5) run before the last exchanges complete, parked
    in SBUF; the last 4 (heads 6,7) land in a fresh bank and are added
    back on DVE.

Emission interleaves B(ch+1) | C(ch) | E(ch-1) weighted by per-unit
PE-time.  Weight/x loads are peeled into separate weave units so every
DMA is issued one unit ahead, and each load class has a dedicated
engine DMA ring (qk->sync, x->scalar/gpsimd, wv->tensor,
yf/wp/out->vector) so loads never queue behind each other.
"""
import sys

sys.path.insert(0, "/opt/trn_rl_repo")

import numpy as np
import ml_dtypes

import concourse.bass as bass
import concourse.tile as tile
from concourse import bacc, mybir
from concourse import bass_utils

F32 = mybir.dt.float32
BF16 = mybir.dt.bfloat16
AF = mybir.ActivationFunctionType
ALU = mybir.AluOpType
BF16NP = ml_dtypes.bfloat16

B, T, C = 4, 2048, 2048
NH, D = 16, 128
HL = 8              # heads per core
NCT = C // 128      # 16 c-tiles
NCH = 4             # 512-token chunks
SCALE = 1.0 / np.sqrt(D)
RG = [[0, 1], [2, 3], [4, 5], [6, 7]]


def _weave(*streams):
    """streams: lists of (fn, weight). Emit round-robin by weighted progress."""
    streams = [s for s in streams if s]
    totals = [sum(w for _, w in s) or 1.0 for s in streams]
    done = [0.0] * len(streams)
    idx = [0] * len(streams)
    while True:
        best, bestv = -1, None
        for i, s in enumerate(streams):
            if idx[i] < len(s):
                v = done[i] / totals[i]
                if bestv is None or v < bestv:
                    best, bestv = i, v
        if best < 0:
            break
        fn, w = streams[best][idx[best]]
        fn()
        done[best] += w
        idx[best] += 1


def _build():
    nc = bacc.Bacc("TRN2", target_bir_lowering=False, debug=False, num_devices=8)
    xb = nc.dram_tensor("xb", [128, NCH, 4, 4, 512], BF16, kind="ExternalInput").ap()
    Wqk = nc.dram_tensor("Wqk", [16, 128, C], BF16, kind="ExternalInput").ap()
    WvT = nc.dram_tensor("WvT", [128, 2, NCT, 512], BF16, kind="ExternalInput").ap()
    Wp2 = nc.dram_tensor("Wp2", [128, 2, 2, 8, 512], BF16, kind="ExternalInput").ap()
    cos2 = nc.dram_tensor("cos2", [128, T], BF16, kind="ExternalInput").ap()
    sin1 = nc.dram_tensor("sin1", [64, T], BF16, kind="ExternalInput").ap()
    sgn = nc.dram_tensor("sgn", [128, 1], F32, kind="ExternalInput").ap()
    m4d = nc.dram_tensor("m4d", [128, 128], BF16, kind="ExternalInput").ap()
    ident = nc.dram_tensor("ident", [128, 128], BF16, kind="ExternalInput").ap()
    out = nc.dram_tensor("out", [T, C // 2], F32, kind="ExternalOutput").ap()

    with tile.TileContext(nc) as tc:
        with tc.tile_pool(name="dram", bufs=1, space="DRAM") as dram, \
             tc.tile_pool(name="const", bufs=1) as cpool:
            ygi = [dram.tile([128, 4, 512], BF16, name=f"ygi{i}") for i in range(7)]
            ygo = [dram.tile([2, 128, 4, 512], BF16, name=f"ygo{i}")
                   for i in range(7)]
            # ch3 exchanges: one quarter (2-head) collective per head pair,
            # staged per-head as each head's attention completes
            ygi3 = [dram.tile([128, 2, 512], BF16, name=f"ygi3_{i}")
                    for i in range(4)]
            ygo3 = [dram.tile([2, 128, 2, 512], BF16, name=f"ygo3_{i}")
                    for i in range(4)]

            xpool = tc.alloc_tile_pool(name="xp", bufs=6)
            wqkp = tc.alloc_tile_pool(name="wqkp", bufs=3)
            wvp = tc.alloc_tile_pool(name="wvp", bufs=1)
            wpp = tc.alloc_tile_pool(name="wpp", bufs=2)
            kp = tc.alloc_tile_pool(name="kp", bufs=32)
            qp = tc.alloc_tile_pool(name="qp", bufs=8)
            vap = tc.alloc_tile_pool(name="vap", bufs=8)
            ebp = tc.alloc_tile_pool(name="ebp", bufs=17)
            yp_ = tc.alloc_tile_pool(name="ypl", bufs=2)
            yfp = tc.alloc_tile_pool(name="yfp", bufs=3)
            abp = tc.alloc_tile_pool(name="abp", bufs=1)
            ynp = tc.alloc_tile_pool(name="ynp", bufs=2)
            op_ = tc.alloc_tile_pool(name="op", bufs=2)
            ps1 = tc.alloc_tile_pool(name="ps1", bufs=2, space="PSUM")
            spp = tc.alloc_tile_pool(name="spp", bufs=4, space="PSUM")
            ypp = tc.alloc_tile_pool(name="ypp", bufs=2, space="PSUM")

            kts = {}   # (h, ch) -> k tile [128, 512]
            qts = {}   # (h, ch) -> q tile [128, 512]
            vas = {}   # (ch, hf) -> va tile [128, 4, 4, 129] (tt, h4, d+1)
            xts = {}   # (ch, qq) -> x tile [128, 4, 512]
            ys = {}    # (ch, hf) -> y tile [128, 4, 512]
            yfs = {}   # (ch, hf) -> yf tile [128, 2, 4, 512] (replica, h4, t)
            wts = {}   # (ch, h, kq) -> prefetched qk weight tile
            wvcur = {}
            wpcur = {}
            ebs = {h: [] for h in range(HL)}
            prts = {}  # (fc, tt) -> partial e3 tile [128, 512] bf16

            m4_sb = cpool.tile([128, 128], BF16)
            id_sb = cpool.tile([128, 128], BF16)
            sg_sb = cpool.tile([128, 1], F32)
            c2_sb = cpool.tile([128, T], BF16)
            s1_sb = cpool.tile([64, T], BF16)

            def load_small_consts():
                nc.gpsimd.dma_start(sg_sb[:], sgn)
                nc.gpsimd.dma_start(m4_sb[:], m4d)
                nc.gpsimd.dma_start(id_sb[:], ident)

            def load_rope_consts():
                nc.scalar.dma_start(c2_sb[:], cos2)
                nc.gpsimd.dma_start(s1_sb[:], sin1)

            def load_x(ch, qq):
                def go():
                    xt = xpool.tile([128, 4, 512], BF16, name="xt")
                    nc.scalar.dma_start(xt[:], xb[:, ch, qq])
                    xts[(ch, qq)] = xt
                return go

            def qk_load(ch, h, kq):
                def go():
                    wt = wqkp.tile([128, C], BF16, name="wt")
                    nc.sync.dma_start(wt[:], Wqk[h * 2 + kq])
                    wts[(ch, h, kq)] = wt
                return go

            def rope_store(ps, ch, h, kq):
                """Rope + bf16 store of a finished q/k PSUM tile."""
                t0 = ch * 512
                a_t = abp.tile([128, 512], BF16, name="a_t")
                nc.vector.tensor_mul(a_t[:], ps[:], c2_sb[:, t0:t0 + 512])
                b_t = abp.tile([128, 512], BF16, name="b_t")
                nc.vector.tensor_mul(
                    b_t[0:64, :], ps[64:128, :], s1_sb[:, t0:t0 + 512])
                nc.vector.tensor_mul(
                    b_t[64:128, :], ps[0:64, :], s1_sb[:, t0:t0 + 512])
                if kq == 0:
                    dst = qp.tile([128, 512], BF16, name="qt")
                    qts[(h, ch)] = dst
                else:
                    dst = kp.tile([128, 512], BF16, name="kt")
                    kts[(h, ch)] = dst
                nc.vector.scalar_tensor_tensor(
                    dst[:], b_t[:], sg_sb[:], a_t[:],
                    op0=ALU.mult, op1=ALU.add)

            def qk_comp(ch, h, kq):
                def go():
                    wt = wts.pop((ch, h, kq))
                    ps = ps1.tile([128, 512], F32, name="ps")
                    for ct in range(NCT):
                        nc.tensor.matmul(
                            ps[:], wt[:, ct * 128:(ct + 1) * 128],
                            xts[(ch, ct // 4)][:, ct % 4, :],
                            start=(ct == 0), stop=(ct == NCT - 1))
                    rope_store(ps, ch, h, kq)
                return go

            def qk_startup():
                """First 4 qk units (h0/h1 x q/k) as x-quarter-interleaved
                waves: 4 open PSUM chains (idle spp pool) so every arriving
                x quarter immediately feeds 16 matmuls.  Loads are spread
                over all three rings in need order; full-tile weight loads
                keep 4KB DMA descriptors (column-sliced loads run 4x
                slower per byte)."""
                # x quarter 0 split by c-tile halves on two rings
                xt0 = xpool.tile([128, 4, 512], BF16, name="xt")
                xts[(0, 0)] = xt0
                w = {}
                w[0] = wqkp.tile([128, C], BF16, name="wt")
                nc.sync.dma_start(w[0][:], Wqk[0])
                w[1] = wqkp.tile([128, C], BF16, name="wt")
                nc.sync.dma_start(w[1][:], Wqk[1])
                nc.scalar.dma_start(xt0[:, 0:2, :], xb[:, 0, 0, 0:2])
                nc.gpsimd.dma_start(xt0[:, 2:4, :], xb[:, 0, 0, 2:4])
                w[2] = wqkp.tile([128, C], BF16, name="wt")
                nc.scalar.dma_start(w[2][:], Wqk[2])
                w[3] = xpool.tile([128, 4, 512], BF16, name="xt")
                nc.gpsimd.dma_start(w[3][:], Wqk[3])
                for qq, eng in ((1, nc.sync), (2, nc.scalar), (3, nc.gpsimd)):
                    xt = xpool.tile([128, 4, 512], BF16, name="xt")
                    eng.dma_start(xt[:], xb[:, 0, qq])
                    xts[(0, qq)] = xt
                load_rope_consts()
                psums = [spp.tile([128, 512], F32, name="sp") for _ in range(4)]

                def wsl(u, ct):
                    if u == 3:   # [128, 4, 512]-shaped borrow
                        return w[3][:, ct // 4, (ct % 4) * 128:(ct % 4 + 1) * 128]
                    return w[u][:, ct * 128:(ct + 1) * 128]

                for p in range(4):
                    for u in range(4):
                        for c4 in range(4):
                            ct = p * 4 + c4
                            nc.tensor.matmul(
                                psums[u][:], wsl(u, ct),
                                xts[(0, p)][:, c4, :],
                                start=(ct == 0), stop=(ct == NCT - 1))
                for u in range(4):
                    rope_store(psums[u], 0, u // 2, u % 2)

            def wv_load(hf):
                def go():
                    wv = wvp.tile([128, NCT, 512], BF16, name="wv")
                    nc.scalar.dma_start(wv[:], WvT[:, hf])
                    wvcur[0] = wv
                return go

            def v_unit(ch, hf, tt):
                def go():
                    if tt == 0:
                        va = vap.tile([128, 4, 4, 129], BF16, name="va")
                        nc.vector.memset(va[:, :, :, 128:129], 1.0)
                        vas[(ch, hf)] = va
                    wv = wvcur[0]
                    va = vas[(ch, hf)]
                    vps = ps1.tile([128, 512], F32, name="ps")
                    for ct in range(NCT):
                        nc.tensor.matmul(
                            vps[:],
                            xts[(ch, ct // 4)][:, ct % 4, tt * 128:(tt + 1) * 128],
                            wv[:, ct, :],
                            start=(ct == 0), stop=(ct == NCT - 1))
                    nc.scalar.copy(
                        va[:, tt, :, 0:128],
                        vps.rearrange("p (h d) -> p h d", d=128))
                return go

            def s_unit(ch, h, j):
                """Scores for 128-token k-block j (global) vs q-chunk ch,
                shrunk to the causal q-range for diagonal blocks."""
                def go():
                    jb = j - 4 * ch          # >=0 only for diagonal blocks
                    qlo = max(0, jb) * 128
                    sp = spp.tile([128, 512], F32, name="sp")
                    nc.tensor.matmul(
                        sp[:, qlo:512],
                        kts[(h, j // 4)][:, (j % 4) * 128:(j % 4 + 1) * 128],
                        qts[(h, ch)][:, qlo:512],
                        start=True, stop=True)
                    eb = ebp.tile([128, 512], BF16, name="eb")
                    nc.scalar.activation(
                        eb[:, qlo:512], sp[:, qlo:512], AF.Exp,
                        scale=float(SCALE))
                    if jb >= 0:
                        nc.vector.tensor_mul(
                            eb[:, qlo:qlo + 128], eb[:, qlo:qlo + 128], m4_sb[:])
                    ebs[h].append(eb)
                return go

            def a_unit(ch, h, ql):
                def go():
                    yt = ypp.tile([128, 129], F32, name="yp", tag="yp")
                    jmax = 4 * ch + ql
                    for j in range(jmax + 1):
                        nc.tensor.matmul(
                            yt[:],
                            ebs[h][j][:, ql * 128:(ql + 1) * 128],
                            vas[(j // 4, h // 4)][:, j % 4, h % 4, :],
                            start=(j == 0), stop=(j == jmax))
                    rc = ynp.tile([128, 1], F32, name="rc")
                    nc.vector.reciprocal(rc[:], yt[:, 128:129])
                    yn = ynp.tile([128, 128], BF16, name="yn")
                    nc.vector.tensor_scalar_mul(yn[:], yt[:, 0:128], rc[:])
                    ytp = ypp.tile([128, 128], BF16, name="ytp", tag="yp")
                    nc.tensor.transpose(ytp[:], yn[:], id_sb[:])
                    nc.vector.tensor_copy(
                        ys[(ch, h // 4)][:, h % 4, ql * 128:(ql + 1) * 128],
                        ytp[:])
                return go

            def y_alloc(ch, hf):
                def go():
                    ys[(ch, hf)] = yp_.tile([128, 4, 512], BF16, name="yc")
                return go

            def d_unit(ch, hf):
                def go():
                    i = ch * 2 + hf
                    nc.gpsimd.dma_start(ygi[i], ys[(ch, hf)][:])
                    nc.gpsimd.collective_compute(
                        "AllGather", ALU.bypass,
                        ins=[ygi[i][:].opt()], outs=[ygo[i][:].opt()],
                        replica_groups=RG)
                return go

            def yf_load(ch, hf):
                def go():
                    yf = yfp.tile([128, 2, 4, 512], BF16, name="yf")
                    nc.gpsimd.dma_start(
                        yf[:], ygo[ch * 2 + hf].rearrange("r p h t -> p r h t"))
                    yfs[(ch, hf)] = yf
                return go

            t31 = {}   # (head, r) -> [128, 512] gathered-ch3-y tile (kp-parked)

            def d3_piece(h):
                """Stage head h's ch3 y into its pair collective input as
                soon as that head's attention finishes."""
                def go():
                    nc.gpsimd.dma_start(
                        ygi3[h // 2][:, h % 2:h % 2 + 1, :],
                        ys[(3, h // 4)][:, h % 4:h % 4 + 1, :])
                return go

            y30 = {}   # hp -> [128, 2, 2, 512] gathered tile (heads 0-3)

            def d3_pair(hp):
                """Exchange ch3 heads (2*hp, 2*hp+1).  Pairs 0,1 (heads
                0-3, consumed by every e3 chain early) land in small yfp
                tiles; pairs 2,3 land in per-(head, r) kp-parked tiles so
                the late chains dep on exactly the data they read."""
                def go():
                    nc.gpsimd.collective_compute(
                        "AllGather", ALU.bypass,
                        ins=[ygi3[hp][:].opt()], outs=[ygo3[hp][:].opt()],
                        replica_groups=RG)
                    if hp < 2:
                        if hp == 0:
                            y30[0] = yfp.tile([128, 2, 4, 512], BF16,
                                              name="yf")
                        nc.gpsimd.dma_start(
                            y30[0][:, :, 2 * hp:2 * hp + 2, :],
                            ygo3[hp].rearrange("r p h t -> p r h t"))
                    else:
                        # pair 3 gates the kernel tail: spread its four
                        # gathered-head loads over all three rings
                        rr = ([nc.gpsimd] * 4 if hp == 2 else
                              [nc.gpsimd, nc.sync, nc.scalar, nc.sync])
                        for i, (hh, r) in enumerate(
                                [(hh, r) for hh in range(2) for r in range(2)]):
                            t = kp.tile([128, 512], BF16, name="kt")
                            rr[i].dma_start(t[:], ygo3[hp][r, :, hh, :])
                            t31[(2 * hp + hh, r)] = t
                return go

            def wp_load(ch, fc):
                """Prefetch proj weights for (ch, fc). For the final
                (ch=3, fc=1) borrow idle x-pool slots so the load never
                waits on fc0's readers."""
                def go():
                    if ch == 3 and fc == 1:
                        for r in range(2):
                            for hh in range(2):
                                wph = xpool.tile([128, 4, 512], BF16, name="xt")
                                (nc.scalar if hh == 0 else nc.gpsimd).dma_start(
                                    wph[:], Wp2[:, r, fc, 4 * hh:4 * hh + 4])
                                wpcur[(fc, r, hh)] = wph
                    else:
                        for r in range(2):
                            wp = wpp.tile([128, 8, 512], BF16, name="wp")
                            nc.gpsimd.dma_start(wp[:], Wp2[:, r, fc])
                            wpcur[(fc, r, 0)] = wp[:, 0:4, :]
                            wpcur[(fc, r, 1)] = wp[:, 4:8, :]
                return go

            def e_unit(ch, fc, tt):
                def go():
                    pp = ps1.tile([128, 512], F32, name="ps")
                    srcs = [(hf, r, h4) for hf in range(2) for r in range(2)
                            for h4 in range(4)]
                    for i, (hf, r, h4) in enumerate(srcs):
                        wp = wpcur[(fc, r, hf)]
                        nc.tensor.matmul(
                            pp[:],
                            yfs[(ch, hf)][:, r, h4, tt * 128:(tt + 1) * 128],
                            wp[:, h4, :],
                            start=(i == 0), stop=(i == 15))
                    ob = op_.tile([128, 512], F32, name="ob")
                    nc.vector.tensor_copy(ob[:], pp[:])
                    t0 = ch * 512 + tt * 128
                    nc.scalar.dma_start(
                        out[t0:t0 + 128, fc * 512:(fc + 1) * 512], ob[:])
                return go

            def e3_early(fc, tt):
                """First 12 matmuls of the (3, fc, tt) chain: heads 0-5
                (hf0 fully + hf1 h4 in {0,1}), parked in SBUF bf16."""
                def go():
                    pp = ps1.tile([128, 512], F32, name="ps")
                    for i, (r, h4) in enumerate(
                            [(r, h4) for r in range(2) for h4 in range(4)]):
                        nc.tensor.matmul(
                            pp[:],
                            y30[0][:, r, h4, tt * 128:(tt + 1) * 128],
                            wpcur[(fc, r, 0)][:, h4, :],
                            start=(i == 0), stop=False)
                    for i, (h4, r) in enumerate(
                            [(h4, r) for h4 in range(2) for r in range(2)]):
                        nc.tensor.matmul(
                            pp[:],
                            t31[(4 + h4, r)][:, tt * 128:(tt + 1) * 128],
                            wpcur[(fc, r, 1)][:, h4, :],
                            start=False, stop=(i == 3))
                    # park the partial in a dead chunk-0 k-tile slot
                    pt = kp.tile([128, 512], BF16, name="kt")
                    nc.vector.tensor_copy(pt[:], pp[:])
                    prts[(fc, tt)] = pt
                return go

            def e3_late(fc, tt, ring):
                """Last 4 matmuls (heads 6,7) + add-back + store."""
                def go():
                    pp = ps1.tile([128, 512], F32, name="ps")
                    srcs = [(r, h4) for h4 in range(2, 4) for r in range(2)]
                    for i, (r, h4) in enumerate(srcs):
                        nc.tensor.matmul(
                            pp[:],
                            t31[(4 + h4, r)][:, tt * 128:(tt + 1) * 128],
                            wpcur[(fc, r, 1)][:, h4, :],
                            start=(i == 0), stop=(i == 3))
                    ob = op_.tile([128, 512], F32, name="ob")
                    nc.vector.scalar_tensor_tensor(
                        ob[:], pp[:], 1.0, prts[(fc, tt)][:],
                        op0=ALU.mult, op1=ALU.add)
                    t0 = 3 * 512 + tt * 128
                    ring.dma_start(
                        out[t0:t0 + 128, fc * 512:(fc + 1) * 512], ob[:])
                return go

            def b_stream(ch, with_xl=None, skip_h01=False):
                """qkv units with loads peeled one unit ahead."""
                pairs = []   # (load_fn_or_None, comp_fn, weight)
                if not skip_h01:
                    for h in (0, 1):
                        for kq in (0, 1):
                            pairs.append(
                                (qk_load(ch, h, kq), qk_comp(ch, h, kq), 3.4))
                pairs.append((wv_load(0), v_unit(ch, 0, 0), 3.4))
                for tt in range(1, 4):
                    pairs.append((None, v_unit(ch, 0, tt), 3.4))
                for h in (2, 3):
                    for kq in (0, 1):
                        pairs.append((qk_load(ch, h, kq), qk_comp(ch, h, kq), 3.4))
                pairs.append((wv_load(1), v_unit(ch, 1, 0), 3.4))
                for tt in range(1, 4):
                    pairs.append((None, v_unit(ch, 1, tt), 3.4))
                for h in (4, 5, 6, 7):
                    for kq in (0, 1):
                        pairs.append((qk_load(ch, h, kq), qk_comp(ch, h, kq), 3.4))
                u = []
                # emit load i+1 before comp i (1-unit lookahead)
                if pairs[0][0] is not None:
                    u.append((pairs[0][0], 0.05))
                for i, (_, comp, w) in enumerate(pairs):
                    nxt = pairs[i + 1][0] if i + 1 < len(pairs) else None
                    if nxt is not None:
                        u.append((nxt, 0.05))
                    u.append((comp, w))
                if with_xl is not None:
                    for qi in range(4):
                        u.append((load_x(with_xl, qi), 0.1))
                return u

            def c_head(ch, h):
                """Units for one head of c_stream(ch)."""
                u = []
                if h % 4 == 0:
                    u.append((y_alloc(ch, h // 4), 0.05))

                def reset(h=h):
                    ebs[h] = []
                u.append((reset, 0.0))
                for j in range(4 * ch + 4):
                    jb = j - 4 * ch
                    w = 0.22 * (512 - max(0, jb) * 128) / 512.0
                    u.append((s_unit(ch, h, j), w))
                for ql in range(4):
                    u.append((a_unit(ch, h, ql), 0.4 + (4 * ch + ql) * 0.066))
                return u

            def c_stream(ch):
                """Attention for chunk ch; also prefetches the e-stage's
                fc0 weights and the first gathered-y half as soon as the
                h0-3 collective has had time to complete."""
                u = []
                for h in range(8):
                    u += c_head(ch, h)
                    if h == 3:
                        u.append((d_unit(ch, 0), 0.1))
                    elif h == 5:
                        u.append((wp_load(ch, 0), 0.1))
                        u.append((yf_load(ch, 0), 0.1))
                    elif h == 7:
                        u.append((d_unit(ch, 1), 0.1))
                return u

            def e_stream(ch):
                u = [(yf_load(ch, 1), 0.1)]
                for tt in range(4):
                    u.append((e_unit(ch, 0, tt), 3.5))
                u.append((wp_load(ch, 1), 0.1))
                for tt in range(4):
                    u.append((e_unit(ch, 1, tt), 3.5))
                return u

            # ---------------- emit ----------------
            load_small_consts()
            qk_startup()

            for fn, _w in b_stream(0, with_xl=1, skip_h01=True):
                fn()

            _weave(b_stream(1, with_xl=2), c_stream(0))
            _weave(b_stream(2, with_xl=3), c_stream(1), e_stream(0))
            _weave(b_stream(3), c_stream(2), e_stream(1))

            # phase 4: c3 heads 0-5 with e_stream(2) + e3 weight loads.
            # Each head's y is staged into its pair-collective input as
            # soon as it finishes; pair exchanges fire at h1/h3/h5 so the
            # CC pipeline drains long before the e3 chains need the data.
            c3_p1 = []
            for h in range(6):
                c3_p1 += c_head(3, h)
                c3_p1.append((d3_piece(h), 0.05))
                if h % 2 == 1:
                    c3_p1.append((d3_pair(h // 2), 0.05))
            e2_plus = e_stream(2)
            e2_plus.append((wp_load(3, 0), 0.1))
            e2_plus.append((wp_load(3, 1), 0.1))
            _weave(c3_p1, e2_plus)

            # phase 5: c3 heads 6,7 run at full priority (so the final
            # exchange fires ASAP); only 3 early-e3 chains are woven in
            # to fill the exp-bound pockets, rest run after.
            c3_p2 = []
            c3_p2 += c_head(3, 6)
            c3_p2.append((d3_piece(6), 0.05))
            c3_p2 += c_head(3, 7)
            c3_p2.append((d3_piece(7), 0.05))
            c3_p2.append((d3_pair(3), 0.05))
            e3e = [(e3_early(fc, tt), 2.6) for fc in range(2) for tt in range(4)]
            _weave(c3_p2, e3e[0:3])
            for fn, w in e3e[3:]:
                fn()

            # tail: last 4 matmuls per chain + add-back, outs spread on rings
            rings = [nc.sync, nc.scalar, nc.gpsimd]
            i = 0
            for fc in range(2):
                for tt in range(4):
                    e3_late(fc, tt, rings[i % 3])()
                    i += 1

            for p in [ypp, spp, ps1, op_, ynp, abp, yfp, yp_, ebp, vap,
                      qp, kp, wpp, wvp, wqkp, xpool]:
                p.release()
    nc.compile()
    return nc


_NC = None


def _get_nc():
    global _NC
    if _NC is None:
        _NC = _build()
    return _NC


def _rope_tables():
    inv_freq = (1.0 / (10000.0 ** (np.arange(0, D, 2, dtype=np.float32) / D)))
    t = np.arange(T, dtype=np.float32)
    freqs = np.outer(t, inv_freq).astype(np.float32)      # [T, 64]
    cos = np.cos(freqs).T                                 # [64, T]
    sin = np.sin(freqs).T
    return cos, sin


def _tile_w(Wt):
    """[128 r, 2048 c] weight tile -> [128 c_lo, 2048 (ct r)] layout."""
    return np.ascontiguousarray(
        Wt.T.reshape(NCT, 128, 128).transpose(1, 0, 2).reshape(128, C))


def make_in_maps(x, W_attn, W_proj):
    perm = np.concatenate([np.arange(0, D, 2), np.arange(1, D, 2)])
    cos, sin = _rope_tables()
    cos2 = np.concatenate([cos, cos], 0).astype(BF16NP)
    sin1 = np.ascontiguousarray(sin).astype(BF16NP)
    sgn = np.concatenate([-np.ones((64, 1)), np.ones((64, 1))]).astype(np.float32)
    p_i = np.arange(128)[:, None]
    c_i = np.arange(128)[None, :]
    m4d_ = (c_i >= p_i).astype(BF16NP)

    xbf = x.astype(BF16NP)
    in_maps = []
    for core in range(8):
        b, g = core // 2, core % 2
        # xb [128 p, ch, qq, ct4, 512 t] = x[b, ch*512+t, (qq*4+ct4)*128+p]
        xt = np.ascontiguousarray(xbf[b].T)               # [C, T]
        xb_ = xt.reshape(4, 4, 128, NCH, 512).transpose(2, 3, 0, 1, 4)
        # q/k weight tiles, rope-permuted; order [h0 q, h0 k, h1 q, ...]
        wtiles = []
        for h in range(HL):
            hg = g * HL + h
            wtiles.append(_tile_w(W_attn[hg * D:(hg + 1) * D][perm]))
            wtiles.append(_tile_w(W_attn[C + hg * D:C + (hg + 1) * D][perm]))
        Wqk_ = np.stack(wtiles, 0).astype(BF16NP)
        # WvT [128 p, half, ct, 512 dv] = Wv[g*1024 + hf*512 + dv, ct*128 + p]
        wv = W_attn[2 * C + g * 1024:2 * C + (g + 1) * 1024]   # [1024 dv, C]
        WvT_ = np.ascontiguousarray(
            wv.reshape(2, 512, NCT, 128).transpose(3, 0, 2, 1)).astype(BF16NP)
        # Wp2 [128 p, r, fc, 8, 512 f] = Wp[g*1024 + fc*512 + f, (r*8+i8)*128+p]
        wp = W_proj[g * 1024:(g + 1) * 1024]                   # [1024 f, C]
        wp_t = wp.reshape(2, 512, NCT, 128)                    # [fc, f, ct, p]
        Wp2_ = np.stack([wp_t[:, :, 0:8], wp_t[:, :, 8:16]], 0)  # [r, fc, f, 8, p]
        Wp2_ = np.ascontiguousarray(Wp2_.transpose(4, 0, 1, 3, 2)).astype(BF16NP)
        in_maps.append({
            "xb": np.ascontiguousarray(xb_),
            "Wqk": Wqk_,
            "WvT": WvT_,
            "Wp2": Wp2_,
            "cos2": cos2, "sin1": sin1, "sgn": sgn,
            "m4d": m4d_, "ident": np.eye(128, dtype=BF16NP),
        })
    return in_maps


def _assemble(results):
    out = np.empty((B, T, C), dtype=np.float32)
    for core in range(8):
        b, g = core // 2, core % 2
        out[b][:, g * (C // 2):(g + 1) * (C // 2)] = results[core]["out"]
    return out


def run(x, W_attn, W_proj, **spmd_kwargs):
    nc = _get_nc()
    in_maps = make_in_maps(np.asarray(x, dtype=np.float32),
                           np.asarray(W_attn, dtype=np.float32),
                           np.asarray(W_proj, dtype=np.float32))
    res = bass_utils.run_bass_kernel_spmd(
        nc, in_maps, core_ids=list(range(8)), **spmd_kwargs)
    return _assemble(res.results), res


def kernel(x, W_attn, W_proj):
    out, _ = run(x, W_attn, W_proj)
    return out
